# revision 1
# baseline (speedup 1.0000x reference)
"""Fused transformer block (QKV proj + attention + FFN + 2x LayerNorm) on 8
Trainium2 NeuronCores.

Sharding: batch (B=2) across two 4-core groups; within a group, tensor
parallel over heads (4 heads / core) for projections+attention, then an
AllToAll switches to row (sequence) sharding for the FFN/LayerNorm tail.

Matmuls run in float32r (full-rate fp32 on the PE array, ~1e-3 rel err);
accumulation is fp32 in PSUM.
"""
import sys

import numpy as np

try:
    import concourse.bass  # noqa: F401
except ImportError:
    sys.path.insert(0, "/opt/trn_rl_repo")

import concourse.bacc as bacc
import concourse.mybir as mybir
import concourse.tile as tile
from concourse import bass_utils
from concourse.masks import make_identity

P = 128
S = 2048          # sequence length (Sq == Sk)
D = 1024          # model dim
H = 16            # total heads
DH = 64           # head dim
NCORES = 8
GROUP = 4         # cores per batch group
JC = D // GROUP   # 256 local projection columns
HL = JC // DH     # 4 local heads
SR = S // GROUP   # 512 output rows per core
DCH = D // P      # 8 d chunks
SCH = S // P      # 16 s chunks
QB = 512          # q block for attention
NQB = S // QB     # 4
F32 = mybir.dt.float32
F32R = mybir.dt.float32r
AF = mybir.ActivationFunctionType
OP = mybir.AluOpType
EPS = 1e-5

_CACHE: dict = {}


def _declare_io(nc):
    t = {}
    t["q"] = nc.dram_tensor("q", [S, D], F32, kind="ExternalInput").ap()
    t["k"] = nc.dram_tensor("k", [S, D], F32, kind="ExternalInput").ap()
    for w in ("wq", "wk", "wv"):
        t[w] = nc.dram_tensor(w, [D, JC], F32, kind="ExternalInput").ap()
    for b in ("bq", "bk", "bv"):
        t[b] = nc.dram_tensor(b, [1, JC], F32, kind="ExternalInput").ap()
    t["wo"] = nc.dram_tensor("wo", [D, D], F32, kind="ExternalInput").ap()
    for b in ("bo", "g0", "b0", "g1", "b1"):
        t[b] = nc.dram_tensor(b, [1, D], F32, kind="ExternalInput").ap()
    t["out"] = nc.dram_tensor("out", [SR, D], F32, kind="ExternalOutput").ap()
    return t


def _transpose_and_project(nc, tc, ctx, pools, x_dram, w_dram, bias_sb, heads_sb,
                           ag_in=None):
    """x [S, D] fp32 DRAM, w [D, JC] -> heads_sb [64, HL, S] f32r (= proj^T,
    head-split, bias added). Optionally DMA the projected slice to ag_in
    ([JC, S] DRAM) for the AllGather."""
    ident = pools["ident_r"]
    xt = pools["xt"].tile([P, DCH, S], F32R, tag="xt")          # x^T, 8 MB
    w_sb = pools["w"].tile([P, DCH, JC], F32R, tag="w")
    nc.gpsimd.dma_start(w_sb[:], w_dram.rearrange("(c p) j -> p c j", p=P))
    # transpose x into xt (PE identity transposes, 128x128 blocks)
    for dc in range(DCH):
        for scg in range(SCH // 4):            # groups of 4 s-blocks / psum bank
            pst = pools["pst"].tile([P, 4 * P], F32R, tag="pst")
            for i in range(4):
                sc = 4 * scg + i
                raw = pools["raw"].tile([P, P], F32R, tag="raw")
                nc.gpsimd.dma_start(
                    raw[:], x_dram[sc * P:(sc + 1) * P, dc * P:(dc + 1) * P])
                nc.tensor.transpose(pst[:, i * P:(i + 1) * P], raw[:], ident)
            nc.vector.tensor_copy(xt[:, dc, 4 * P * scg:4 * P * (scg + 1)], pst[:])
    # project: out^T [JC, S] = w^T x^T, accumulate over d chunks
    for jc2 in range(JC // P):
        for nb in range(S // QB):
            ps = pools["psp"].tile([P, QB], F32, tag="psp")
            for dc in range(DCH):
                nc.tensor.matmul(
                    ps[:], w_sb[:, dc, jc2 * P:(jc2 + 1) * P],
                    xt[:, dc, nb * QB:(nb + 1) * QB],
                    start=(dc == 0), stop=(dc == DCH - 1))
            for hh in range(2):
                h = 2 * jc2 + hh
                nc.vector.tensor_scalar(
                    out=heads_sb[:, h, nb * QB:(nb + 1) * QB],
                    in0=ps[hh * DH:(hh + 1) * DH, :],
                    scalar1=bias_sb[:, h:h + 1], scalar2=None, op0=OP.add)
    if ag_in is not None:
        for h in range(HL):
            nc.sync.dma_start(
                ag_in[h * DH:(h + 1) * DH, :],
                heads_sb[:, h, :].bitcast(F32))


def _emit(nc, tc, ctx, t):
    # SBUF pools are a LIFO stack: open order must be reverse of close order.
    pools = {}
    pools["const"] = ctx.enter_context(tc.tile_pool(name="const", bufs=1))
    heads_cm = tc.tile_pool(name="heads", bufs=1)
    heads = heads_cm.__enter__()
    w_cm = tc.tile_pool(name="w", bufs=2)
    pools["w"] = w_cm.__enter__()
    raw_cm = tc.tile_pool(name="raw", bufs=6)
    pools["raw"] = raw_cm.__enter__()
    xt_cm = tc.tile_pool(name="xt", bufs=1)
    pools["xt"] = xt_cm.__enter__()
    pools["pst"] = ctx.enter_context(tc.tile_pool(name="pst", bufs=2, space="PSUM"))
    psp_cm = tc.tile_pool(name="psp", bufs=3, space="PSUM")
    pools["psp"] = psp_cm.__enter__()
    dram = ctx.enter_context(tc.tile_pool(name="dram", bufs=1, space="DRAM"))
    const = pools["const"]

    # constants
    ident_f = const.tile([P, P], F32)
    make_identity(nc, ident_f)
    ident_r = const.tile([P, P], F32R)
    nc.vector.tensor_copy(ident_r[:], ident_f[:])
    pools["ident_r"] = ident_r
    eps_t = const.tile([P, 1], F32)
    nc.vector.memset(eps_t, EPS)

    # per-partition bias views [64, HL]
    bq_sb = const.tile([DH, HL], F32)
    nc.sync.dma_start(bq_sb[:], t["bq"].rearrange("o (h p) -> (o p) h", p=DH))
    bk_sb = const.tile([DH, HL], F32)
    nc.sync.dma_start(bk_sb[:], t["bk"].rearrange("o (h p) -> (o p) h", p=DH))
    # broadcast params [128, N]
    bvb = const.tile([P, JC], F32)
    nc.gpsimd.dma_start(bvb[:], t["bv"].to_broadcast([P, JC]))
    bob = const.tile([P, D], F32)
    nc.gpsimd.dma_start(bob[:], t["bo"].to_broadcast([P, D]))
    g0b = const.tile([P, D], F32)
    nc.gpsimd.dma_start(g0b[:], t["g0"].to_broadcast([P, D]))
    b0b = const.tile([P, D], F32)
    nc.gpsimd.dma_start(b0b[:], t["b0"].to_broadcast([P, D]))
    g1b = const.tile([P, D], F32)
    nc.gpsimd.dma_start(g1b[:], t["g1"].to_broadcast([P, D]))
    b1b = const.tile([P, D], F32)
    nc.gpsimd.dma_start(b1b[:], t["b1"].to_broadcast([P, D]))

    groups = [list(range(GROUP)), list(range(GROUP, 2 * GROUP))]

    # ---- K path: K^T, Kp^T (own 4 heads), AllGather of Kp^T ----
    k_heads = heads.tile([DH, HL, S], F32R)           # Kp^T per local head
    ag_in = dram.tile([JC, S], F32)
    ag_out = dram.tile([D, S], F32)
    _transpose_and_project(nc, tc, ctx, pools, t["k"], t["wk"], bk_sb, k_heads,
                           ag_in=ag_in)
    nc.gpsimd.collective_compute(
        "AllGather", OP.bypass, ins=[ag_in.opt()], outs=[ag_out.opt()],
        replica_groups=groups)

    # ---- Q path ----
    q_heads = heads.tile([DH, HL, S], F32R)
    _transpose_and_project(nc, tc, ctx, pools, t["q"], t["wq"], bq_sb, q_heads)
    xt_cm.__exit__(None, None, None)          # free x^T (64 KB/partition)

    # ---- Vp natural [S, JC] with fused ones column: [128, SCH, HL, 65] ----
    vp = heads.tile([P, SCH, HL, DH + 1], F32R)
    wv_sb = pools["w"].tile([P, DCH, JC], F32R, tag="w")
    nc.gpsimd.dma_start(wv_sb[:], t["wv"].rearrange("(c p) j -> p c j", p=P))
    for sc in range(SCH):
        psv = pools["psp"].tile([P, JC], F32, tag="psp")
        for dc in range(DCH):
            kpf = pools["raw"].tile([P, P], F32R, tag="raw")
            nc.sync.dma_start(
                kpf[:], ag_out[dc * P:(dc + 1) * P, sc * P:(sc + 1) * P]
                .bitcast(F32R))
            nc.tensor.matmul(psv[:], kpf[:], wv_sb[:, dc, :],
                             start=(dc == 0), stop=(dc == DCH - 1))
        nc.vector.tensor_tensor(
            out=vp[:, sc, :, 0:DH],
            in0=psv.rearrange("p (h d) -> p h d", h=HL),
            in1=bvb.rearrange("p (h d) -> p h d", h=HL), op=OP.add)
    ones_t = const.tile([P, 1], F32)
    nc.vector.memset(ones_t, 1.0)
    nc.vector.tensor_copy(
        vp[:, :, :, DH:DH + 1],
        ones_t[:, None, :].broadcast_to([P, SCH, HL, 1]))
    raw_cm.__exit__(None, None, None)         # free block-load tiles
    w_cm.__exit__(None, None, None)           # free weight slices
    psp_cm.__exit__(None, None, None)         # free projection psum banks

    # ---- attention: per (head, q-block) ----
    att_cm = tc.tile_pool(name="att", bufs=1)
    att = att_cm.__enter__()
    oh = att.tile([DH, HL, S], F32)                   # (Qh + attnV)^T unnormed->final
    spool_cm = tc.tile_pool(name="spool", bufs=4)
    spool = spool_cm.__enter__()
    epool_cm = tc.tile_pool(name="epool", bufs=3)
    epool = epool_cm.__enter__()
    ps_s = ctx.enter_context(tc.tile_pool(name="ps_s", bufs=2, space="PSUM"))
    ps_a_cm = tc.tile_pool(name="ps_a", bufs=2, space="PSUM")
    ps_a = ps_a_cm.__enter__()
    for h in range(HL):
        for qb in range(NQB):
            qsl = slice(qb * QB, (qb + 1) * QB)
            psA = ps_a.tile([DH + 1, QB], F32, tag="psA")
            for g in range(SCH // 2):
                psS = ps_s.tile([P, 2 * QB], F32, tag="psS")
                for i in range(2):
                    kc = 2 * g + i
                    nc.tensor.matmul(
                        psS[:, i * QB:(i + 1) * QB],
                        k_heads[:, h, kc * P:(kc + 1) * P],
                        q_heads[:, h, qsl], start=True, stop=True)
                e_sb = epool.tile([P, 2 * QB], F32R, tag="e")
                nc.scalar.activation(e_sb[:], psS[:], AF.Exp, scale=0.125)
                for i in range(2):
                    kc = 2 * g + i
                    nc.tensor.matmul(
                        psA[:], vp[:, kc, h, :], e_sb[:, i * QB:(i + 1) * QB],
                        start=(kc == 0), stop=(kc == SCH - 1))
            recip = spool.tile([1, QB], F32, tag="recip")
            nc.vector.reciprocal(recip[:], psA[DH:DH + 1, :])
            recipb = spool.tile([DH, QB], F32, tag="recipb")
            nc.gpsimd.partition_broadcast(recipb[:], recip[:], channels=DH)
            nc.vector.tensor_tensor(out=oh[:, h, qsl], in0=psA[0:DH, :],
                                    in1=recipb[:], op=OP.mult)
            nc.vector.tensor_tensor(out=oh[:, h, qsl], in0=oh[:, h, qsl],
                                    in1=q_heads[:, h, qsl], op=OP.add)

    # ---- transpose heads to natural rows, AllToAll to row sharding ----
    a2a_in = dram.tile([S, JC], F32)
    a2a_out = dram.tile([S, JC], F32)
    for sc in range(SCH):
        psT = pools["pst"].tile([P, JC], F32, tag="pst")
        for h in range(HL):
            nc.tensor.transpose(psT[:, h * DH:(h + 1) * DH],
                                oh[:, h, sc * P:(sc + 1) * P],
                                ident_f[0:DH, 0:DH])
        stg = spool.tile([P, JC], F32, tag="stg")
        nc.vector.tensor_copy(stg[:], psT[:])
        nc.sync.dma_start(a2a_in[sc * P:(sc + 1) * P, :], stg[:])
    nc.gpsimd.collective_compute(
        "AllToAll", OP.bypass, ins=[a2a_in.opt()], outs=[a2a_out.opt()],
        replica_groups=[list(range(NCORES))])
    ps_a_cm.__exit__(None, None, None)
    epool_cm.__exit__(None, None, None)
    spool_cm.__exit__(None, None, None)
    att_cm.__exit__(None, None, None)         # free oh (32 KB)
    heads_cm.__exit__(None, None, None)       # free k/q heads + vp (80 KB)

    # ---- stage 2: rows [SR, D] : LN0 -> FFN(Wo)+gelu+residual -> LN1 ----
    s2 = ctx.enter_context(tc.tile_pool(name="s2", bufs=1))
    ln_tmp = ctx.enter_context(tc.tile_pool(name="ln_tmp", bufs=4))
    NS2 = SR // P                                     # 4 row chunks
    # 8-rank AllToAll: shard p of a2a_out = rows [256c:256c+256) x cols
    # [256(p%4):...) of batch p//4. Chunks 0,1 -> batch 0; chunks 2,3 -> b 1.
    o_sb = s2.tile([P, NS2, D], F32)
    for sc2 in range(NS2):
        bb, rr = divmod(sc2, 2)
        for j in range(GROUP):
            pr = bb * GROUP + j
            base = pr * (S // NCORES) + rr * P
            nc.sync.dma_start(
                o_sb[:, sc2, j * JC:(j + 1) * JC],
                a2a_out[base:base + P, :])

    def layernorm(src_ap, dst_ap, gb, bb, sc2):
        """src [128, D] -> dst [128, D] layernorm with broadcast gamma/beta."""
        red = ln_tmp.tile([P, 1], F32, tag="red")
        nc.vector.tensor_reduce(red[:], src_ap, mybir.AxisListType.X, OP.add)
        negmean = ln_tmp.tile([P, 1], F32, tag="negmean")
        nc.vector.tensor_scalar_mul(negmean[:], red[:], -1.0 / D)
        sq = ln_tmp.tile([P, D], F32, tag="sq")
        sumsq = ln_tmp.tile([P, 1], F32, tag="sumsq")
        nc.scalar.activation(sq[:], src_ap, AF.Square, bias=negmean[:],
                             scale=1.0, accum_out=sumsq[:])
        std = ln_tmp.tile([P, 1], F32, tag="std")
        nc.scalar.activation(std[:], sumsq[:], AF.Sqrt, bias=eps_t[:],
                             scale=1.0 / D)
        rstd = ln_tmp.tile([P, 1], F32, tag="rstd")
        nc.vector.reciprocal(rstd[:], std[:])
        nc.vector.tensor_scalar(out=dst_ap, in0=src_ap, scalar1=negmean[:],
                                scalar2=rstd[:], op0=OP.add, op1=OP.mult)
        nc.vector.tensor_tensor(out=dst_ap, in0=dst_ap, in1=gb[:], op=OP.mult)
        nc.vector.tensor_tensor(out=dst_ap, in0=dst_ap, in1=bb[:], op=OP.add)

    ln0 = s2.tile([P, NS2, D], F32R)
    for sc2 in range(NS2):
        layernorm(o_sb[:, sc2, :], ln0[:, sc2, :], g0b, b0b, sc2)

    # transpose ln0 -> [128, DCH, SR] for the Wo contraction
    ln0t = s2.tile([P, DCH, SR], F32R)
    for dc in range(DCH):
        psL = pools["pst"].tile([P, SR], F32R, tag="pst")
        for sc2 in range(NS2):
            nc.tensor.transpose(psL[:, sc2 * P:(sc2 + 1) * P],
                                ln0[:, sc2, dc * P:(dc + 1) * P], ident_r)
        nc.vector.tensor_copy(ln0t[:, dc, :], psL[:])

    wo_sb = s2.tile([P, DCH, D], F32R)
    nc.gpsimd.dma_start(wo_sb[:], t["wo"].rearrange("(c p) j -> p c j", p=P))
    o2 = s2.tile([P, NS2, D], F32)
    for sc2 in range(NS2):
        psF = ps_s.tile([P, D], F32, tag="psS")
        for dc in range(DCH):
            for nb in range(2):
                nc.tensor.matmul(
                    psF[:, nb * QB:(nb + 1) * QB],
                    ln0t[:, dc, sc2 * P:(sc2 + 1) * P],
                    wo_sb[:, dc, nb * QB:(nb + 1) * QB],
                    start=(dc == 0), stop=(dc == DCH - 1))
        fb = ln_tmp.tile([P, D], F32, tag="fb")
        nc.vector.tensor_tensor(out=fb[:], in0=psF[:], in1=bob[:], op=OP.add)
        gel = ln_tmp.tile([P, D], F32, tag="gel")
        nc.scalar.activation(gel[:], fb[:], AF.Gelu)
        nc.vector.tensor_tensor(out=o2[:, sc2, :], in0=ln0[:, sc2, :],
                                in1=gel[:], op=OP.add)

    for sc2 in range(NS2):
        fin = ln_tmp.tile([P, D], F32, tag="fin")
        layernorm(o2[:, sc2, :], fin[:], g1b, b1b, sc2)
        nc.sync.dma_start(t["out"][sc2 * P:(sc2 + 1) * P, :], fin[:])


def build():
    if "nc" in _CACHE:
        return _CACHE["nc"]
    from contextlib import ExitStack
    nc = bacc.Bacc("TRN2", target_bir_lowering=False, debug=False,
                   num_devices=NCORES)
    t = _declare_io(nc)
    with tile.TileContext(nc) as tc:
        with ExitStack() as ctx:
            _emit(nc, tc, ctx, t)
    nc.compile()
    _CACHE["nc"] = nc
    return nc


def make_in_maps(Q, K, Wq, bq, Wk, bk, Wv, bv, Wo, bo, g0, b0, g1, b1):
    in_maps = []
    for c in range(NCORES):
        b, g = divmod(c, GROUP)
        jsl = slice(g * JC, (g + 1) * JC)
        ac = np.ascontiguousarray
        in_maps.append({
            "q": ac(Q[b]), "k": ac(K[b]),
            "wq": ac(Wq[:, jsl]), "wk": ac(Wk[:, jsl]), "wv": ac(Wv[:, jsl]),
            "bq": ac(bq[jsl].reshape(1, JC)), "bk": ac(bk[jsl].reshape(1, JC)),
            "bv": ac(bv[jsl].reshape(1, JC)),
            "wo": ac(Wo), "bo": ac(bo.reshape(1, D)),
            "g0": ac(g0.reshape(1, D)), "b0": ac(b0.reshape(1, D)),
            "g1": ac(g1.reshape(1, D)), "b1": ac(b1.reshape(1, D)),
        })
    return in_maps


def run(in_maps, trace=False, **kwargs):
    nc = build()
    return bass_utils.run_bass_kernel_spmd(
        nc, in_maps, core_ids=list(range(NCORES)), trace=trace, **kwargs)


def kernel(**inputs):
    inputs = {k: np.asarray(v, dtype=np.float32) for k, v in inputs.items()}
    in_maps = make_in_maps(
        inputs["Q"], inputs["K"], inputs["Wq"], inputs["bq"], inputs["Wk"],
        inputs["bk"], inputs["Wv"], inputs["bv"], inputs["Wo"], inputs["bo"],
        inputs["g0"], inputs["b0"], inputs["g1"], inputs["b1"])
    res = run(in_maps, trace=False)
    B = 2
    RS = S // NCORES  # 256 rows of each batch per core
    out = np.empty((B, S, D), dtype=np.float32)
    for c in range(NCORES):
        r = res.results[c]["out"]  # [512, D]: rows 0-255 -> b0, 256-511 -> b1
        out[0, c * RS:(c + 1) * RS, :] = r[:RS]
        out[1, c * RS:(c + 1) * RS, :] = r[RS:]
    return out


if __name__ == "__main__":
    rng = np.random.default_rng(0)
    ins = {n: rng.standard_normal(s).astype(np.float32) * (0.03125 if n.startswith("w") else 1.0)
           for n, s in [("Q", (2, S, D)), ("K", (2, S, D)), ("Wq", (D, D)),
                        ("Wk", (D, D)), ("Wv", (D, D)), ("Wo", (D, D))]}
    for n in ("bq", "bk", "bv", "bo", "b0", "b1"):
        ins[n] = np.zeros(D, np.float32)
    for n in ("g0", "g1"):
        ins[n] = np.ones(D, np.float32)
    out = kernel(**ins)
    print("ran ok", out.shape, out.dtype)



# revision 12
# speedup vs baseline: 1.8275x; 1.8275x over previous
"""Fused transformer block (QKV proj + attention + FFN + 2x LayerNorm) on 8
Trainium2 NeuronCores.

Sharding: batch (B=2) across two 4-core groups; within a group, tensor
parallel over heads (4 heads / core) for projections+attention, then an
AllToAll switches to row (sequence) sharding for the FFN/LayerNorm tail.

v2 design notes (vs the f32r baseline):
- Host pre-transposes Q/K and pre-packs every tensor partition-major in
  bf16, so there are no on-device input transposes and every DMA line is
  partition-contiguous.
- The AllGather of Kp^T is gone: Vp = Kp@Wv = K@(Wk@Wv) + (bk@Wv + bv),
  with Wkv fused on device from a host-supplied Wk^T (layout-only prep).
- All big matmuls run in bf16 (1 cycle/row on the PE vs 2 for f32r).
- attn@V runs in fp8e4 with DoubleRow perf mode (2 rows/cycle), with the
  softmax denominator fused in as a ones-column of V.
- exp() is split across the scalar (Act) engine and the DVE/GpSimd
  engines; the latter two use a Schraudolph bit-trick exp (~3% rel err,
  harmless under softmax) since only the Act engine has native Exp.
- Softmax normalization + Q residual happen in natural layout after a
  PE transpose of the PSUM attention output, killing the [1,512]
  reciprocals and partition broadcasts of the baseline.
"""
import sys

import numpy as np

try:
    import concourse.bass  # noqa: F401
except ImportError:
    sys.path.insert(0, "/opt/trn_rl_repo")

import ml_dtypes

import concourse.bacc as bacc
import concourse.mybir as mybir
import concourse.tile as tile
from concourse import bass_utils
from concourse.masks import make_identity

P = 128
S = 2048          # sequence length (Sq == Sk)
D = 1024          # model dim
H = 16            # total heads
DH = 64           # head dim
NCORES = 8
GROUP = 4         # cores per batch group
JC = D // GROUP   # 256 local projection columns
HL = JC // DH     # 4 local heads
DCH = D // P      # 8 d chunks
SCH = S // P      # 16 s chunks
QB = 512          # q block for attention
NQB = S // QB     # 4
SR2 = 2 * S // NCORES  # 512 output rows per core (256 per batch)

F32 = mybir.dt.float32
BF16 = mybir.dt.bfloat16
FP8 = mybir.dt.float8e4
I32 = mybir.dt.int32
AF = mybir.ActivationFunctionType
OP = mybir.AluOpType
DR = mybir.MatmulPerfMode.DoubleRow
EPS = 1e-5

# Schraudolph fast-exp constants: exp(y) ~= bitcast_f32(i32(y*EXA + EXB))
# calibrated for truncation, max rel err ~3.0% over y in [-14, 6].
EXA = 12102203.161561485        # 2^23 / ln(2)
EXB = float((127 << 23) - 366400)
# softmax shift: exp(s*0.125 - SM_SHIFT) keeps e well under the fp8e4 max of
# 240 (values >= ~272 become inf) for rows with large ||q||; softmax is
# invariant to the shift since the ones-column denominator scales equally.
SM_SHIFT = 4.5
# engine per g-chunk of each attention unit (8 chunks of [128,1024] exps).
# GPSIMD cannot read PSUM, so only the Act engine (native Exp) and the DVE
# (Schraudolph bit-trick) participate.
EXP_ENGINES = ("dve", "dve", "dve", "act", "act", "act", "act", "act")

NPBF16 = ml_dtypes.bfloat16

_CACHE: dict = {}


def _declare_io(nc):
    t = {}
    t["qt"] = nc.dram_tensor("qt", [P, DCH * S], BF16, kind="ExternalInput").ap()
    t["kt"] = nc.dram_tensor("kt", [P, DCH * S], BF16, kind="ExternalInput").ap()
    for w in ("wq", "wk", "wv"):
        t[w] = nc.dram_tensor(w, [P, DCH * JC], BF16, kind="ExternalInput").ap()
    t["wkt"] = nc.dram_tensor("wkt", [P, DCH * D], BF16, kind="ExternalInput").ap()
    t["wo"] = nc.dram_tensor("wo", [P, DCH * D], BF16, kind="ExternalInput").ap()
    t["bq_h"] = nc.dram_tensor("bq_h", [DH, HL], F32, kind="ExternalInput").ap()
    t["bk_h"] = nc.dram_tensor("bk_h", [DH, HL], F32, kind="ExternalInput").ap()
    t["bkc"] = nc.dram_tensor("bkc", [P, DCH], BF16, kind="ExternalInput").ap()
    t["bv_s"] = nc.dram_tensor("bv_s", [1, JC], F32, kind="ExternalInput").ap()
    for b in ("bo", "g0", "b0", "g1", "b1"):
        t[b] = nc.dram_tensor(b, [1, D], F32, kind="ExternalInput").ap()
    t["out"] = nc.dram_tensor("out", [SR2, D], F32, kind="ExternalOutput").ap()
    return t


def _emit(nc, tc, ctx, t):
    # ---- psum pools (persist; 4 + 2 + 2 = 8 banks) ----
    ps_s = ctx.enter_context(tc.tile_pool(name="ps_s", bufs=2, space="PSUM"))
    ps_a = ctx.enter_context(tc.tile_pool(name="ps_a", bufs=2, space="PSUM"))
    ps_t = ctx.enter_context(tc.tile_pool(name="ps_t", bufs=2, space="PSUM"))
    dram = ctx.enter_context(tc.tile_pool(name="dram", bufs=1, space="DRAM"))

    const = ctx.enter_context(tc.tile_pool(name="const", bufs=1))

    # ---- constants / small params ----
    ident_f = const.tile([P, P], F32)
    make_identity(nc, ident_f)
    ident_b = const.tile([P, P], BF16)
    nc.vector.tensor_copy(ident_b[:], ident_f[:])
    eps_t = const.tile([P, 1], F32)
    nc.vector.memset(eps_t, EPS)
    smshift_t = const.tile([P, 1], F32)
    nc.vector.memset(smshift_t, -SM_SHIFT)

    bq_sb = const.tile([DH, HL], F32)
    nc.sync.dma_start(bq_sb[:], t["bq_h"])
    bk_sb = const.tile([DH, HL], F32)
    nc.sync.dma_start(bk_sb[:], t["bk_h"])
    bkc_sb = const.tile([P, DCH], BF16)
    nc.sync.dma_start(bkc_sb[:], t["bkc"])
    bv_sb = const.tile([1, JC], F32)
    nc.sync.dma_start(bv_sb[:], t["bv_s"])

    brow = {}
    bcast = {}
    for b in ("bo", "g0", "b0", "g1", "b1"):
        brow[b] = const.tile([1, D], F32, name=f"brow_{b}")
        nc.sync.dma_start(brow[b][:], t[b])
        bcast[b] = const.tile([P, D], F32, name=f"bcast_{b}")
        nc.gpsimd.partition_broadcast(bcast[b][:], brow[b][:], channels=P)

    # ---- persistent activations ----
    heads_cm = tc.tile_pool(name="heads", bufs=1)
    heads = heads_cm.__enter__()
    k_heads = heads.tile([DH, HL, S], BF16)
    q_heads = heads.tile([DH, HL, S], BF16)
    # V padded to 80 cols: 0:64 = V, 64 = ones (softmax denominator), 65:80
    # zero pad: the DoubleRow ldweights AP requires the k-tile stride to be
    # 16-byte aligned. k-tile pairs are adjacent (dim 3).
    vp = heads.tile([P, SCH // 2, HL, 2, DH + 16], FP8)
    oh_nat = heads.tile([P, SCH, JC], BF16)

    # ---- weights + transposed inputs (freed before attention) ----
    wx_cm = tc.tile_pool(name="wx", bufs=1)
    wx = wx_cm.__enter__()
    wkt_sb = wx.tile([P, DCH, D], BF16)
    nc.sync.dma_start(wkt_sb[:], t["wkt"].rearrange("p (c n) -> p c n", c=DCH))
    wv_sb = wx.tile([P, DCH, JC], BF16)
    nc.sync.dma_start(wv_sb[:], t["wv"].rearrange("p (c n) -> p c n", c=DCH))
    wk_sb = wx.tile([P, DCH, JC], BF16)
    nc.sync.dma_start(wk_sb[:], t["wk"].rearrange("p (c n) -> p c n", c=DCH))
    wq_sb = wx.tile([P, DCH, JC], BF16)
    nc.sync.dma_start(wq_sb[:], t["wq"].rearrange("p (c n) -> p c n", c=DCH))
    wkv_sb = wx.tile([P, DCH, JC], BF16)

    x_cm = tc.tile_pool(name="x", bufs=1)
    xp = x_cm.__enter__()
    kt_sb = xp.tile([P, DCH, S], BF16)
    nc.sync.dma_start(kt_sb[:], t["kt"].rearrange("p (c n) -> p c n", c=DCH))
    qt_sb = xp.tile([P, DCH, S], BF16)
    nc.sync.dma_start(qt_sb[:], t["qt"].rearrange("p (c n) -> p c n", c=DCH))

    # ---- Wkv = Wk @ Wv (local JC columns); vp bias = bk @ Wv + bv ----
    for dc in range(DCH):
        psW = ps_s.tile([P, JC], F32, tag="ps_s")
        for ec in range(DCH):
            nc.tensor.matmul(psW[:], wkt_sb[:, ec, dc * P:(dc + 1) * P],
                             wv_sb[:, ec, :], start=(ec == 0), stop=(ec == DCH - 1))
        nc.vector.tensor_copy(wkv_sb[:, dc, :], psW[:])
    psB = ps_t.tile([1, JC], F32, tag="ps_t")
    for ec in range(DCH):
        nc.tensor.matmul(psB[:], bkc_sb[:, ec:ec + 1], wv_sb[:, ec, :],
                         start=(ec == 0), stop=(ec == DCH - 1))
    vpb_row = const.tile([1, JC], F32)
    nc.vector.tensor_tensor(out=vpb_row[:], in0=psB[:], in1=bv_sb[:], op=OP.add)
    vpb = const.tile([P, JC], F32)
    nc.gpsimd.partition_broadcast(vpb[:], vpb_row[:], channels=P)

    # ---- Kp^T / Qp^T projections: heads[j, s] = sum_d W[d, j] X^T[d, s] ----
    def project(w_sb, x_sb, bias_sb, dst):
        for jc2 in range(JC // P):          # 2 head-pairs
            for nb in range(S // QB):       # 4 s-blocks
                ps = ps_s.tile([P, QB], F32, tag="ps_s")
                for dc in range(DCH):
                    nc.tensor.matmul(
                        ps[:], w_sb[:, dc, jc2 * P:(jc2 + 1) * P],
                        x_sb[:, dc, nb * QB:(nb + 1) * QB],
                        start=(dc == 0), stop=(dc == DCH - 1))
                for hh in range(2):
                    h = 2 * jc2 + hh
                    nc.vector.tensor_scalar(
                        out=dst[:, h, nb * QB:(nb + 1) * QB],
                        in0=ps[hh * DH:(hh + 1) * DH, :],
                        scalar1=bias_sb[:, h:h + 1], scalar2=None, op0=OP.add)

    project(wk_sb, kt_sb, bk_sb, k_heads)

    # ---- Vp natural [s, j] = sum_d K^T[d, s]^T Wkv[d, j], + bias, fp8 ----
    for sc in range(SCH):
        psV = ps_s.tile([P, JC], F32, tag="ps_s")
        for dc in range(DCH):
            nc.tensor.matmul(psV[:], kt_sb[:, dc, sc * P:(sc + 1) * P],
                             wkv_sb[:, dc, :], start=(dc == 0), stop=(dc == DCH - 1))
        g2, i2 = divmod(sc, 2)
        nc.vector.tensor_tensor(
            out=vp[:, g2, :, i2, 0:DH],
            in0=psV.rearrange("p (h d) -> p h d", h=HL),
            in1=vpb.rearrange("p (h d) -> p h d", h=HL), op=OP.add)
    nc.vector.memset(vp[:, :, :, :, DH:DH + 1], 1.0)
    nc.vector.memset(vp[:, :, :, :, DH + 1:DH + 16], 0.0)

    project(wq_sb, qt_sb, bq_sb, q_heads)

    x_cm.__exit__(None, None, None)        # free kt/qt (64 KB/part)
    wx_cm.__exit__(None, None, None)       # free weights (32 KB/part)

    # ---- attention ----
    att_cm = tc.tile_pool(name="att", bufs=1)
    att = att_cm.__enter__()
    epool_cm = tc.tile_pool(name="epool", bufs=3)
    epool = epool_cm.__enter__()
    ipool_cm = tc.tile_pool(name="ipool", bufs=2)
    ipool = ipool_cm.__enter__()
    opool_cm = tc.tile_pool(name="opool", bufs=2)
    opool = opool_cm.__enter__()

    a2a_in = dram.tile([S, JC], BF16)
    a2a_out = dram.tile([S, JC], BF16)

    NG = SCH // 2                           # 8 kc-pair groups
    for qb in range(NQB):
        qsl = slice(qb * QB, (qb + 1) * QB)
        for h in range(HL):
            psA = ps_a.tile([DH + 16, QB], F32, tag="ps_a")
            for g in range(NG):
                psS = ps_s.tile([P, 2 * QB], F32, tag="ps_s")
                for i in range(2):
                    kc = 2 * g + i
                    nc.tensor.matmul(
                        psS[:, i * QB:(i + 1) * QB],
                        k_heads[:, h, kc * P:(kc + 1) * P],
                        q_heads[:, h, qsl], start=True, stop=True)
                e_sb = epool.tile([P, 2, QB], FP8, tag="e")
                eng = EXP_ENGINES[g]
                if eng == "act":
                    nc.scalar.activation(
                        e_sb.rearrange("p a b -> p (a b)"), psS[:], AF.Exp,
                        scale=0.125, bias=smshift_t[:])
                else:
                    veng = nc.vector if eng == "dve" else nc.gpsimd
                    i32_sb = ipool.tile([P, 2 * QB], I32, tag="i32")
                    veng.tensor_scalar(
                        out=i32_sb[:], in0=psS[:], scalar1=EXA * 0.125,
                        scalar2=EXB - SM_SHIFT * EXA, op0=OP.mult, op1=OP.add)
                    veng.tensor_copy(
                        e_sb.rearrange("p a b -> p (a b)"), i32_sb.bitcast(F32))
                nc.tensor.matmul(
                    psA[:], vp[:, g, h, :, :], e_sb[:],
                    start=(g == 0), stop=(g == NG - 1), perf_mode=DR)
            # copy to sbuf, transpose to natural, normalize + residual
            oht = opool.tile([DH + 1, QB], BF16, tag="oht")
            nc.vector.tensor_copy(oht[:], psA[0:DH + 1, :])
            pst = ps_t.tile([P, NQB, 2 * DH + 2], BF16, tag="ps_t")
            for qc in range(NQB):
                nc.tensor.transpose(
                    pst[:, qc, 0:DH + 1],
                    oht[:, qc * P:(qc + 1) * P], ident_b[0:DH + 1, 0:DH + 1])
                nc.tensor.transpose(
                    pst[:, qc, DH + 2:2 * DH + 2],
                    q_heads[:, h, (qb * NQB + qc) * P:(qb * NQB + qc + 1) * P],
                    ident_b[0:DH, 0:DH])
            rec = opool.tile([P, NQB, 1], F32, tag="rec")
            nc.vector.reciprocal(rec[:], pst[:, :, DH:DH + 1])
            for qc in range(NQB):
                sc = qb * NQB + qc
                nc.vector.tensor_scalar(
                    out=oh_nat[:, sc, h * DH:(h + 1) * DH],
                    in0=pst[:, qc, 0:DH], scalar1=rec[:, qc, :],
                    scalar2=None, op0=OP.mult)
            nc.vector.tensor_tensor(
                out=oh_nat[:, qb * NQB:(qb + 1) * NQB, h * DH:(h + 1) * DH],
                in0=oh_nat[:, qb * NQB:(qb + 1) * NQB, h * DH:(h + 1) * DH],
                in1=pst[:, :, DH + 2:2 * DH + 2], op=OP.add)
        for qc in range(NQB):
            sc = qb * NQB + qc
            nc.sync.dma_start(a2a_in[sc * P:(sc + 1) * P, :], oh_nat[:, sc, :])

    nc.gpsimd.collective_compute(
        "AllToAll", OP.bypass, ins=[a2a_in.opt()], outs=[a2a_out.opt()],
        replica_groups=[list(range(NCORES))])

    opool_cm.__exit__(None, None, None)
    ipool_cm.__exit__(None, None, None)
    epool_cm.__exit__(None, None, None)
    att_cm.__exit__(None, None, None)
    heads_cm.__exit__(None, None, None)

    # ---- stage 2: rows [SR2, D] : LN0 -> Wo+gelu+residual -> LN1 ----
    s2 = ctx.enter_context(tc.tile_pool(name="s2", bufs=1))
    ln_tmp = ctx.enter_context(tc.tile_pool(name="ln_tmp", bufs=4))
    NS2 = SR2 // P                                    # 4 row chunks
    wo_sb = s2.tile([P, DCH, D], BF16)
    nc.sync.dma_start(wo_sb[:], t["wo"].rearrange("p (c n) -> p c n", c=DCH))
    # 8-rank AllToAll: shard p of a2a_out = rows [256c:256c+256) x cols
    # [256(p%4):...) of batch p//4. Chunks 0,1 -> batch 0; chunks 2,3 -> b 1.
    o_sb = s2.tile([P, NS2, D], BF16)
    for sc2 in range(NS2):
        bb, rr = divmod(sc2, 2)
        for j in range(GROUP):
            pr = bb * GROUP + j
            base = pr * (S // NCORES) + rr * P
            nc.sync.dma_start(
                o_sb[:, sc2, j * JC:(j + 1) * JC],
                a2a_out[base:base + P, :])

    def layernorm(src_ap, dst_ap, gb, bb):
        """src [128, D] -> dst [128, D] layernorm with broadcast gamma/beta."""
        red = ln_tmp.tile([P, 1], F32, tag="red")
        nc.vector.tensor_reduce(red[:], src_ap, mybir.AxisListType.X, OP.add)
        negmean = ln_tmp.tile([P, 1], F32, tag="negmean")
        nc.vector.tensor_scalar_mul(negmean[:], red[:], -1.0 / D)
        sq = ln_tmp.tile([P, D], BF16, tag="sq")
        sumsq = ln_tmp.tile([P, 1], F32, tag="sumsq")
        nc.scalar.activation(sq[:], src_ap, AF.Square, bias=negmean[:],
                             scale=1.0, accum_out=sumsq[:])
        std = ln_tmp.tile([P, 1], F32, tag="std")
        nc.scalar.activation(std[:], sumsq[:], AF.Sqrt, bias=eps_t[:],
                             scale=1.0 / D)
        rstd = ln_tmp.tile([P, 1], F32, tag="rstd")
        nc.vector.reciprocal(rstd[:], std[:])
        nc.vector.tensor_scalar(out=dst_ap, in0=src_ap, scalar1=negmean[:],
                                scalar2=rstd[:], op0=OP.add, op1=OP.mult)
        nc.vector.tensor_tensor(out=dst_ap, in0=dst_ap, in1=gb[:], op=OP.mult)
        nc.vector.tensor_tensor(out=dst_ap, in0=dst_ap, in1=bb[:], op=OP.add)

    ln0 = s2.tile([P, NS2, D], BF16)
    for sc2 in range(NS2):
        layernorm(o_sb[:, sc2, :], ln0[:, sc2, :], bcast["g0"], bcast["b0"])

    # transpose ln0 -> [128, DCH, SR2] for the Wo contraction
    ln0t = s2.tile([P, DCH, SR2], BF16)
    for dc in range(DCH):
        psL = ps_a.tile([P, SR2], BF16, tag="ps_a")
        for sc2 in range(NS2):
            nc.tensor.transpose(psL[:, sc2 * P:(sc2 + 1) * P],
                                ln0[:, sc2, dc * P:(dc + 1) * P], ident_b)
        nc.vector.tensor_copy(ln0t[:, dc, :], psL[:])

    o2 = s2.tile([P, NS2, D], BF16)
    for sc2 in range(NS2):
        psF = ps_s.tile([P, D], F32, tag="ps_s")
        for dc in range(DCH):
            for nb in range(2):
                nc.tensor.matmul(
                    psF[:, nb * QB:(nb + 1) * QB],
                    ln0t[:, dc, sc2 * P:(sc2 + 1) * P],
                    wo_sb[:, dc, nb * QB:(nb + 1) * QB],
                    start=(dc == 0), stop=(dc == DCH - 1))
        fb = ln_tmp.tile([P, D], F32, tag="fb")
        nc.vector.tensor_tensor(out=fb[:], in0=psF[:], in1=bcast["bo"][:],
                                op=OP.add)
        gel = ln_tmp.tile([P, D], BF16, tag="gel")
        nc.scalar.activation(gel[:], fb[:], AF.Gelu)
        nc.vector.tensor_tensor(out=o2[:, sc2, :], in0=ln0[:, sc2, :],
                                in1=gel[:], op=OP.add)

    for sc2 in range(NS2):
        fin = ln_tmp.tile([P, D], F32, tag="fin")
        layernorm(o2[:, sc2, :], fin[:], bcast["g1"], bcast["b1"])
        nc.sync.dma_start(t["out"][sc2 * P:(sc2 + 1) * P, :], fin[:])


def build():
    if "nc" in _CACHE:
        return _CACHE["nc"]
    from contextlib import ExitStack
    nc = bacc.Bacc("TRN2", target_bir_lowering=False, debug=False,
                   num_devices=NCORES)
    t = _declare_io(nc)
    with tile.TileContext(nc) as tc:
        with ExitStack() as ctx:
            _emit(nc, tc, ctx, t)
    nc.compile()
    _CACHE["nc"] = nc
    return nc


def _pmajor(a):
    """[D0, N] with D0 = c*128+p  ->  [128, c*N] partition-major bf16."""
    d0, n = a.shape
    c = d0 // P
    return np.ascontiguousarray(
        a.reshape(c, P, n).transpose(1, 0, 2).reshape(P, c * n).astype(NPBF16))


def make_in_maps(Q, K, Wq, bq, Wk, bk, Wv, bv, Wo, bo, g0, b0, g1, b1):
    f32 = np.float32
    in_maps = []
    wkt_full = _pmajor(np.ascontiguousarray(Wk.T))
    wo_full = _pmajor(Wo)
    bkc = np.ascontiguousarray(
        bk.reshape(DCH, P).T.astype(NPBF16))          # [128, DCH]
    qt = {}
    kt = {}
    for b in range(2):
        qt[b] = _pmajor(np.ascontiguousarray(Q[b].T))
        kt[b] = _pmajor(np.ascontiguousarray(K[b].T))
    for c in range(NCORES):
        b, g = divmod(c, GROUP)
        jsl = slice(g * JC, (g + 1) * JC)
        in_maps.append({
            "qt": qt[b], "kt": kt[b],
            "wq": _pmajor(Wq[:, jsl]), "wk": _pmajor(Wk[:, jsl]),
            "wv": _pmajor(Wv[:, jsl]),
            "wkt": wkt_full, "wo": wo_full,
            "bq_h": np.ascontiguousarray(
                bq[jsl].reshape(HL, DH).T.astype(f32)),
            "bk_h": np.ascontiguousarray(
                bk[jsl].reshape(HL, DH).T.astype(f32)),
            "bkc": bkc,
            "bv_s": np.ascontiguousarray(bv[jsl].reshape(1, JC).astype(f32)),
            "bo": np.ascontiguousarray(bo.reshape(1, D).astype(f32)),
            "g0": np.ascontiguousarray(g0.reshape(1, D).astype(f32)),
            "b0": np.ascontiguousarray(b0.reshape(1, D).astype(f32)),
            "g1": np.ascontiguousarray(g1.reshape(1, D).astype(f32)),
            "b1": np.ascontiguousarray(b1.reshape(1, D).astype(f32)),
        })
    return in_maps


def run(in_maps, trace=False, **kwargs):
    nc = build()
    return bass_utils.run_bass_kernel_spmd(
        nc, in_maps, core_ids=list(range(NCORES)), trace=trace, **kwargs)


def kernel(**inputs):
    inputs = {k: np.asarray(v, dtype=np.float32) for k, v in inputs.items()}
    in_maps = make_in_maps(
        inputs["Q"], inputs["K"], inputs["Wq"], inputs["bq"], inputs["Wk"],
        inputs["bk"], inputs["Wv"], inputs["bv"], inputs["Wo"], inputs["bo"],
        inputs["g0"], inputs["b0"], inputs["g1"], inputs["b1"])
    res = run(in_maps, trace=False)
    B = 2
    RS = S // NCORES  # 256 rows of each batch per core
    out = np.empty((B, S, D), dtype=np.float32)
    for c in range(NCORES):
        r = res.results[c]["out"]  # [512, D]: rows 0-255 -> b0, 256-511 -> b1
        out[0, c * RS:(c + 1) * RS, :] = r[:RS]
        out[1, c * RS:(c + 1) * RS, :] = r[RS:]
    return out


if __name__ == "__main__":
    rng = np.random.default_rng(0)
    ins = {n: rng.standard_normal(s).astype(np.float32) * (0.03125 if n.startswith("W") else 1.0)
           for n, s in [("Q", (2, S, D)), ("K", (2, S, D)), ("Wq", (D, D)),
                        ("Wk", (D, D)), ("Wv", (D, D)), ("Wo", (D, D))]}
    for n in ("bq", "bk", "bv", "bo", "b0", "b1"):
        ins[n] = np.zeros(D, np.float32)
    for n in ("g0", "g1"):
        ins[n] = np.ones(D, np.float32)
    out = kernel(**ins)
    print("ran ok", out.shape, out.dtype)
